# revision 13
# baseline (speedup 1.0000x reference)
"""Trainium2 Bass kernel for BiDAF-style bidirectional attention (v6).

Reference computation (per batch element n; M=1 folded away):
    s[i,j]  = h[i].w_h + u[j].w_u + (h[i]*u[j]).w_hu + b      [JX, JQ]
    a_u     = softmax_j(s);     u_a[i] = sum_j a_u[i,j] u[j]   (c2q)
    a_h     = softmax_i(max_j s);  h_a = sum_i a_h[i] h[i]     (q2c)
    out     = concat(h, u_a, h*u_a, h*h_a)                     [JX, 4D]

Sharding: data-parallel over batch N=8, one NeuronCore per batch element.
alpha_b drops out (both softmaxes are shift-invariant); accepted but unused.

v6 design (bf16 I/O, host-folded weights -- see _prep_inputs):
  - hT uploaded BLOCK-major so both hT DMAs are DRAM-contiguous; they ride
    the Sync HW queue alone (nothing else competes for early reads).  h is
    on the GpSimd queue but dep-gated on the first score matmul so its 1MB
    never starves the critical hT stream.  aux on the Scalar queue.
  - 5-matmul bf16 warmup bridges to hT-b0 arrival -> HAM opens ~11.5us.
  - Scores sT[j,i] per 512-block: 4 bf16 matmuls; ET = exp(sT + uwu[j]).
  - c2q: u_a tiles through a 4-deep PSUM rotation (reusing the freed score
    banks); o2 = u_a/z evictions split Scalar ACT x6 / DVE tensor_scalar
    x2; o3 = o2*h as 2-tile-wide bf16 2x TTs on DVE only.
  - q2c: all 8 ET re-transposes into ONE PSUM bank; per-block j-max on
    DVE; zsum via N=1 PE matmuls; hap/zq/haT on PE; ha_row on Scalar
    straight from PSUM; o4T[d,i] = hT*hacol via 4x-mode DVE tensor_scalar
    (3D APs bridge block-major hT -> chunk-major o4T staging).
  - Output slab-major [P, 4*NT*D]; writes spread across Sync (u_a, h*u_a),
    GpSimd (pass, o4T hi) and Scalar (o4T lo) queues, all contiguous.
"""

import numpy as np

N_B, M_B, JX, JQ, D = 8, 1, 1024, 128, 512
P = 128
NT = JX // P    # 8 i-tiles
KC = D // P     # 4 d-chunks
IB = 512        # i-block width for score matmuls
NB = JX // IB   # 2 blocks
TPB = NT // NB  # 4 tiles per block

_CACHE = {}


def _build_program():
    from contextlib import ExitStack

    import concourse.bass as bass
    import concourse.tile as tile
    from concourse import bacc, mybir
    from concourse.masks import make_identity
    from concourse.tile_rust import add_dep_helper

    f32 = mybir.dt.float32
    bf16 = mybir.dt.bfloat16
    EXP = mybir.ActivationFunctionType.Exp
    AX = mybir.AxisListType.X
    ds = bass.ds

    nc = bacc.Bacc("TRN2", target_bir_lowering=False, debug=False, num_devices=8)
    h_d = nc.dram_tensor("hrows", [P, NT * D], bf16, kind="ExternalInput").ap()
    # block-major: hT_d[p, b*(KC*IB) + k*IB + i'] = h[b*IB+i', k*128+p]
    hT_d = nc.dram_tensor("hT", [P, NB * KC * IB], bf16, kind="ExternalInput").ap()
    aux_d = nc.dram_tensor("aux", [P, 2 * D], bf16, kind="ExternalInput").ap()
    uwu_d = nc.dram_tensor("uwu", [P, 1], f32, kind="ExternalInput").ap()
    # slabs: 0=h rows, 1=u_a rows, 2=h*u_a rows, 3=o4T chunk-major
    out_d = nc.dram_tensor("out", [P, 4 * NT * D], bf16, kind="ExternalOutput").ap()

    with tile.TileContext(nc) as tc, ExitStack() as ctx:
        consts = ctx.enter_context(tc.tile_pool(name="consts", bufs=1))
        stage = ctx.enter_context(tc.tile_pool(name="stage", bufs=1))
        # PSUM budget (8 banks): acc=1, s0=2(reused by late ups), tp=1,
        # ua=2, hap=1  (+1 spare)
        ps = ctx.enter_context(tc.tile_pool(name="ps", bufs=2, space="PSUM"))

        # ---- input DMAs ----
        hT = consts.tile([P, NB * KC * IB], bf16)   # block-major, like DRAM
        for b in range(NB):
            sl = ds(b * KC * IB, KC * IB)
            nc.sync.dma_start(hT[:, sl], hT_d[:, sl])
        uwu = consts.tile([P, 1], f32)
        nc.gpsimd.dma_start(uwu[:], uwu_d[:])
        h_all = consts.tile([P, NT * D], bf16)      # tile t: h[t*128+p, d]
        h_dma = nc.gpsimd.dma_start(h_all[:], h_d[:])
        aux = consts.tile([P, 2 * D], bf16)         # [u | uwbT]
        nc.scalar.dma_start(aux[:], aux_d[:])
        u_sb = aux[:, ds(0, D)]
        uwbT = aux[:, ds(D, D)]
        ident = consts.tile([P, P], bf16)
        make_identity(nc, ident[:])                 # gpsimd affine_select

        # ---- constants ----
        warm = consts.tile([P, D], bf16)
        nc.vector.memset(warm[:], 0.25)
        ones_col = consts.tile([P, 1], bf16)
        nc.vector.memset(ones_col[:], 1.0)
        one1 = consts.tile([1, 1], bf16)
        nc.vector.memset(one1[:], 1.0)

        # ---- PE warmup: opens the HAM clock gate while input DMAs fly ----
        wp = ps.tile([P, D], f32, tag="acc", bufs=1)
        for _ in range(5):
            nc.tensor.matmul(wp[:], warm[:, ds(0, P)], warm[:], start=True, stop=True)

        # ---- working tiles ----
        ET = consts.tile([JQ, JX], bf16)
        m_exp = consts.tile([P, NT], f32)
        m_bf = consts.tile([P, NT], bf16)
        z_rec = consts.tile([P, NT], f32)
        hap = ps.tile([1, D], f32, tag="hap", bufs=1)
        ua_blk = [
            stage.tile([P, TPB * D], bf16, tag=f"ua{b}", name=f"ua_blk{b}")
            for b in range(NB)
        ]
        o3_blk = [
            stage.tile([P, TPB * D], bf16, tag=f"o3{b}", name=f"o3_blk{b}")
            for b in range(NB)
        ]
        o4T = consts.tile([P, KC * JX], bf16)       # chunk-major

        # ---- scores + exp per block ----
        first_mm = None
        sps = []
        for b in range(NB):
            sp = ps.tile([JQ, IB], f32, tag="s0")
            for k in range(KC):
                mm = nc.tensor.matmul(
                    sp[:], uwbT[:, ds(k * JQ, JQ)],
                    hT[:, ds(b * KC * IB + k * IB, IB)],
                    start=(k == 0), stop=(k == KC - 1),
                )
                if first_mm is None:
                    first_mm = mm
            sps.append(sp)
        # hold the 1MB h load out of the critical hT/aux read window
        add_dep_helper(h_dma.ins, first_mm.ins, sync=True,
                       reason="delay h load until hT/aux are resident")
        for b in range(NB):
            nc.scalar.activation(ET[:, ds(b * IB, IB)], sps[b][:], EXP, bias=uwu[:])

        # ---- ET re-transposes (one PSUM bank) + per-block max/zsum ----
        zcol = ps.tile([P, NT], f32, tag="acc", bufs=1)
        et = ps.tile([P, NT * P], bf16, tag="tp", bufs=1)
        for b in range(NB):
            for q in range(TPB):
                t = b * TPB + q
                nc.tensor.transpose(et[:, ds(t * P, P)], ET[:, ds(t * P, P)], ident[:])
            for q in range(TPB):
                t = b * TPB + q
                nc.tensor.matmul(
                    zcol[:, ds(t, 1)], ET[:, ds(t * P, P)], ones_col[:],
                    start=True, stop=True, skip_group_check=True,
                )
            sl = ds(b * TPB, TPB)
            nc.vector.reciprocal(z_rec[:, sl], zcol[:, sl])
            et3 = et[:].rearrange("p (t x) -> p t x", t=NT)
            nc.vector.reduce_max(m_exp[:, sl], et3[:, ds(b * TPB, TPB), :], axis=AX)
            nc.gpsimd.tensor_copy(m_bf[:, sl], m_exp[:, sl])

        # ---- passthrough (slab 0) per half as h lands ----
        for b in range(NB):
            nc.gpsimd.dma_start(
                out_d[:, ds(b * TPB * D, TPB * D)], h_all[:, ds(b * TPB * D, TPB * D)]
            )

        # ---- c2q u_a b0 (4-deep PSUM rotation); q2c hap/zq; u_a b1 ----
        up_tags = ["ua", "ua", "s0", "s0", "ua", "ua", "ux", "s0"]
        ups = []
        for t in range(TPB):
            up = ps.tile([P, D], f32, tag=up_tags[t], name=f"up{t}")
            nc.tensor.matmul(up[:], ET[:, ds(t * P, P)], u_sb, start=True, stop=True)
            ups.append(up)
        for t in range(NT):
            nc.tensor.matmul(
                hap[:], m_bf[:, ds(t, 1)], h_all[:, ds(t * D, D)],
                start=(t == 0), stop=(t == NT - 1), skip_group_check=True,
            )
        mrow = consts.tile([P, 1], f32)
        nc.vector.reduce_sum(mrow[:], m_exp[:], axis=AX)
        mrow_bf = consts.tile([P, 1], bf16)
        nc.gpsimd.tensor_copy(mrow_bf[:], mrow[:])
        zqp = ps.tile([1, 1], f32, tag="acc", bufs=1)
        nc.tensor.matmul(zqp[:], mrow_bf[:], ones_col[:], start=True, stop=True)
        for t in range(TPB, NT):
            up = ps.tile([P, D], f32, tag=up_tags[t], name=f"up{t}", bufs=1 if t == 6 else 2)
            nc.tensor.matmul(up[:], ET[:, ds(t * P, P)], u_sb, start=True, stop=True)
            ups.append(up)

        # ---- evictions + q2c tail, FIFO-choreographed per engine ----
        def o2_of(t):
            b, q = divmod(t, TPB)
            return ua_blk[b][:, ds(q * D, D)]

        def o3_pair(pair):
            b, qp = divmod(pair, TPB // 2)
            nc.vector.tensor_mul(
                o3_blk[b][:, ds(qp * 2 * D, 2 * D)],
                ua_blk[b][:, ds(qp * 2 * D, 2 * D)],
                h_all[:, ds(pair * 2 * D, 2 * D)],
            )

        # scalar: o2 t0, t1 first; vector: mrow already queued
        nc.scalar.mul(o2_of(0), ups[0][:], z_rec[:, ds(0, 1)])
        nc.scalar.mul(o2_of(1), ups[1][:], z_rec[:, ds(1, 1)])
        o3_pair(0)                       # vector, fires when o2 t1 done
        rzq = consts.tile([1, 1], f32)
        nc.vector.reciprocal(rzq[:], zqp[:])
        ha_row = consts.tile([1, D], bf16)
        nc.scalar.mul(ha_row[:], hap[:], rzq[:])   # scalar, after o2 t1
        haT = ps.tile([P, KC], f32, tag="acc", bufs=1)
        for k in range(KC):
            nc.tensor.matmul(           # PE, after ua t4-7
                haT[:, ds(k, 1)], ha_row[:, ds(k * P, P)], one1[:],
                start=True, stop=True, skip_group_check=True,
            )
        nc.scalar.mul(o2_of(2), ups[2][:], z_rec[:, ds(2, 1)])
        nc.scalar.mul(o2_of(3), ups[3][:], z_rec[:, ds(3, 1)])
        nc.vector.tensor_scalar_mul(o2_of(5), ups[5][:], z_rec[:, ds(5, 1)])
        nc.vector.tensor_scalar_mul(o2_of(7), ups[7][:], z_rec[:, ds(7, 1)])
        o3_pair(1)                       # vector
        nc.sync.dma_start(out_d[:, ds(NT * D, TPB * D)], ua_blk[0][:])
        nc.sync.dma_start(out_d[:, ds(2 * NT * D, TPB * D)], o3_blk[0][:])
        nc.scalar.mul(o2_of(4), ups[4][:], z_rec[:, ds(4, 1)])
        nc.scalar.mul(o2_of(6), ups[6][:], z_rec[:, ds(6, 1)])
        # vector: hacol then o4T (4x tensor_scalar, 3D APs) then o3 b1
        hacol = consts.tile([P, KC], f32)
        nc.vector.tensor_copy(hacol[:], haT[:])
        hT4 = hT[:].rearrange("p (b k x) -> p b k x", b=NB, k=KC)
        o4T4 = o4T[:].rearrange("p (k b x) -> p k b x", k=KC, b=NB)
        for k in range(KC):
            nc.vector.tensor_scalar_mul(
                o4T4[:, k], hT4[:, :, k, :], hacol[:, ds(k, 1)]
            )
        nc.gpsimd.dma_start(out_d[:, ds(3 * NT * D, 2 * JX)], o4T[:, ds(0, 2 * JX)])
        o3_pair(2)                       # vector
        nc.gpsimd.dma_start(
            out_d[:, ds(3 * NT * D + 2 * JX, 2 * JX)], o4T[:, ds(2 * JX, 2 * JX)]
        )
        nc.sync.dma_start(out_d[:, ds((NT + TPB) * D, TPB * D)], ua_blk[1][:])
        o3_pair(3)                       # vector
        nc.sync.dma_start(out_d[:, ds((2 * NT + TPB) * D, TPB * D)], o3_blk[1][:])

    nc.compile()
    return nc


def _get_nc():
    if "nc" not in _CACHE:
        _CACHE["nc"] = _build_program()
    return _CACHE["nc"]


def _ensure_axon_hooks_stub():
    import sys
    import types

    try:
        import antenv.axon_hooks  # noqa: F401
    except ImportError:
        mod = types.ModuleType("antenv.axon_hooks")
        _hook = [None]
        mod.set_axon_ntff_profile_hook = lambda hook: _hook.__setitem__(0, hook)
        mod.get_axon_ntff_profile_hook = lambda: _hook[0]
        sys.modules["antenv.axon_hooks"] = mod


def _prep_inputs(h, u, alpha_w):
    """Host-side layout/weight prep (data movement + O(JQ*D) weight folding)."""
    import ml_dtypes

    bf = ml_dtypes.bfloat16
    w_h, w_u, w_hu = alpha_w[:D], alpha_w[D:2 * D], alpha_w[2 * D:]
    in_maps = []
    for n in range(N_B):
        hn = h[n]                                   # [JX, D] f32
        un = u[n]                                   # [JQ, D] f32
        hrows = np.ascontiguousarray(
            hn.reshape(NT, P, D).transpose(1, 0, 2).reshape(P, NT * D)
        ).astype(bf)
        # block-major hT: [P, (b, k, i')]
        hT = np.ascontiguousarray(
            hn.T.reshape(KC, P, NB, IB).transpose(1, 2, 0, 3).reshape(P, NB * KC * IB)
        ).astype(bf)
        uwb = un * w_hu[None, :] + w_h[None, :]     # [JQ, D]
        uwbT = uwb.T.reshape(KC, P, JQ).transpose(1, 0, 2).reshape(P, KC * JQ)
        aux = np.concatenate([un, uwbT], axis=1).astype(bf)
        uwu = (un @ w_u).reshape(P, 1).astype(np.float32)
        in_maps.append({"hrows": hrows, "hT": hT, "aux": np.ascontiguousarray(aux),
                        "uwu": uwu})
    return in_maps


def _decode_out(res):
    outs = []
    for n in range(N_B):
        o = np.asarray(res.results[n]["out"]).astype(np.float32)
        slabs = o.reshape(P, 4, NT * D)
        rows = slabs[:, :3, :].reshape(P, 3, NT, D).transpose(2, 0, 1, 3)  # [NT,P,3,D]
        o4 = slabs[:, 3, :].reshape(P, KC, JX).transpose(2, 1, 0)          # [JX,KC,P]
        full = np.concatenate(
            [rows.reshape(JX, 3 * D), o4.reshape(JX, D)], axis=1
        )
        outs.append(full)
    return np.stack(outs, axis=0).reshape(N_B, M_B, JX, 4 * D)


def kernel(h, u, alpha_w, alpha_b=None, **_unused):
    _ensure_axon_hooks_stub()
    from concourse.bass_utils import run_bass_kernel_spmd

    h = np.ascontiguousarray(np.asarray(h, dtype=np.float32)).reshape(N_B, JX, D)
    u = np.ascontiguousarray(np.asarray(u, dtype=np.float32)).reshape(N_B, JQ, D)
    alpha_w = np.ascontiguousarray(np.asarray(alpha_w, dtype=np.float32)).reshape(3 * D)

    nc = _get_nc()
    in_maps = _prep_inputs(h, u, alpha_w)
    res = run_bass_kernel_spmd(nc, in_maps, core_ids=list(range(N_B)))
    return _decode_out(res)


# revision 18
# speedup vs baseline: 1.1376x; 1.1376x over previous
"""Trainium2 Bass kernel for BiDAF-style bidirectional attention (v6).

Reference computation (per batch element n; M=1 folded away):
    s[i,j]  = h[i].w_h + u[j].w_u + (h[i]*u[j]).w_hu + b      [JX, JQ]
    a_u     = softmax_j(s);     u_a[i] = sum_j a_u[i,j] u[j]   (c2q)
    a_h     = softmax_i(max_j s);  h_a = sum_i a_h[i] h[i]     (q2c)
    out     = concat(h, u_a, h*u_a, h*h_a)                     [JX, 4D]

Sharding: data-parallel over batch N=8, one NeuronCore per batch element.
alpha_b drops out (both softmaxes are shift-invariant); accepted but unused.

v6 design (bf16 I/O, host-folded weights -- see _prep_inputs):
  - hT uploaded BLOCK-major so both hT DMAs are DRAM-contiguous; they ride
    the Sync HW queue alone (nothing else competes for early reads).  h is
    on the GpSimd queue but dep-gated on the first score matmul so its 1MB
    never starves the critical hT stream.  aux on the Scalar queue.
  - 5-matmul bf16 warmup bridges to hT-b0 arrival -> HAM opens ~11.5us.
  - Scores sT[j,i] per 512-block: 4 bf16 matmuls; ET = exp(sT + uwu[j]).
  - c2q: u_a tiles through a 4-deep PSUM rotation (reusing the freed score
    banks); o2 = u_a/z evictions split Scalar ACT x6 / DVE tensor_scalar
    x2; o3 = o2*h as 2-tile-wide bf16 2x TTs on DVE only.
  - q2c: all 8 ET re-transposes into ONE PSUM bank; per-block j-max on
    DVE; zsum via N=1 PE matmuls; hap/zq/haT on PE; ha_row on Scalar
    straight from PSUM; o4T[d,i] = hT*hacol via 4x-mode DVE tensor_scalar
    (3D APs bridge block-major hT -> chunk-major o4T staging).
  - Output slab-major [P, 4*NT*D]; writes spread across Sync (u_a, h*u_a),
    GpSimd (pass, o4T hi) and Scalar (o4T lo) queues, all contiguous.
"""

import numpy as np

N_B, M_B, JX, JQ, D = 8, 1, 1024, 128, 512
P = 128
NT = JX // P    # 8 i-tiles
KC = D // P     # 4 d-chunks
IB = 512        # i-block width for score matmuls
NB = JX // IB   # 2 blocks
TPB = NT // NB  # 4 tiles per block

_CACHE = {}


def _build_program():
    from contextlib import ExitStack

    import concourse.bass as bass
    import concourse.tile as tile
    from concourse import bacc, mybir
    from concourse.masks import make_identity
    from concourse.tile_rust import add_dep_helper

    f32 = mybir.dt.float32
    bf16 = mybir.dt.bfloat16
    EXP = mybir.ActivationFunctionType.Exp
    AX = mybir.AxisListType.X
    ds = bass.ds

    nc = bacc.Bacc("TRN2", target_bir_lowering=False, debug=False, num_devices=8)
    h_d = nc.dram_tensor("hrows", [P, NT * D], bf16, kind="ExternalInput").ap()
    # block-major: hT_d[p, b*(KC*IB) + k*IB + i'] = h[b*IB+i', k*128+p]
    hT_d = nc.dram_tensor("hT", [P, NB * KC * IB], bf16, kind="ExternalInput").ap()
    aux_d = nc.dram_tensor("aux", [P, 2 * D], bf16, kind="ExternalInput").ap()
    uwu_d = nc.dram_tensor("uwu", [P, 1], f32, kind="ExternalInput").ap()
    # slabs: 0=h rows, 1=u_a rows, 2=h*u_a rows, 3=o4T chunk-major
    out_d = nc.dram_tensor("out", [P, 4 * NT * D], bf16, kind="ExternalOutput").ap()

    with tile.TileContext(nc) as tc, ExitStack() as ctx:
        consts = ctx.enter_context(tc.tile_pool(name="consts", bufs=1))
        stage = ctx.enter_context(tc.tile_pool(name="stage", bufs=1))
        # PSUM budget (8 banks): acc=1, s0=2(reused by late ups), tp=1,
        # ua=2, hap=1  (+1 spare)
        ps = ctx.enter_context(tc.tile_pool(name="ps", bufs=2, space="PSUM"))

        # ---- input DMAs ----
        hT = consts.tile([P, NB * KC * IB], bf16)   # block-major, like DRAM
        for b in range(NB):
            sl = ds(b * KC * IB, KC * IB)
            nc.sync.dma_start(hT[:, sl], hT_d[:, sl])
        uwu = consts.tile([P, 1], f32)
        nc.gpsimd.dma_start(uwu[:], uwu_d[:])
        h_all = consts.tile([P, NT * D], bf16)      # tile t: h[t*128+p, d]
        h_dma = nc.gpsimd.dma_start(h_all[:], h_d[:])
        aux = consts.tile([P, 2 * D], bf16)         # [u | uwbT]
        nc.scalar.dma_start(aux[:], aux_d[:])
        u_sb = aux[:, ds(0, D)]
        uwbT = aux[:, ds(D, D)]
        ident = consts.tile([P, P], bf16)
        make_identity(nc, ident[:])                 # gpsimd affine_select

        # ---- constants ----
        warm = consts.tile([P, D], bf16)
        nc.vector.memset(warm[:], 0.25)
        ones_col = consts.tile([P, 1], bf16)
        nc.vector.memset(ones_col[:], 1.0)
        one1 = consts.tile([1, 1], bf16)
        nc.vector.memset(one1[:], 1.0)

        # ---- PE warmup: opens the HAM clock gate while input DMAs fly.
        # 7 matmuls bridge from PE start (~8.4us) to hT-b0 arrival (~13us)
        # so the PE never idles long enough to re-throttle. ----
        wp = ps.tile([P, D], f32, tag="acc", bufs=1)
        warm_mms = [
            nc.tensor.matmul(wp[:], warm[:, ds(0, P)], warm[:], start=True, stop=True)
            for _ in range(7)
        ]

        # ---- working tiles ----
        ET = consts.tile([JQ, JX], bf16)
        m_exp = consts.tile([P, NT], f32)
        m_bf = consts.tile([P, NT], bf16)
        z_rec = consts.tile([P, NT], f32)
        hap = ps.tile([1, D], f32, tag="hap", bufs=1)
        ua_blk = [
            stage.tile([P, TPB * D], bf16, tag=f"ua{b}", name=f"ua_blk{b}")
            for b in range(NB)
        ]
        o3_blk = [
            stage.tile([P, TPB * D], bf16, tag=f"o3{b}", name=f"o3_blk{b}")
            for b in range(NB)
        ]
        o4T = consts.tile([P, KC * JX], bf16)       # chunk-major

        # ---- scores + exp per block ----
        first_mm = None
        sps = []
        for b in range(NB):
            sp = ps.tile([JQ, IB], f32, tag="s0")
            for k in range(KC):
                mm = nc.tensor.matmul(
                    sp[:], uwbT[:, ds(k * JQ, JQ)],
                    hT[:, ds(b * KC * IB + k * IB, IB)],
                    start=(k == 0), stop=(k == KC - 1),
                )
                if first_mm is None:
                    first_mm = mm
            sps.append(sp)
        # hold the 1MB h load out of the critical hT/aux read window
        add_dep_helper(h_dma.ins, warm_mms[2].ins, sync=True,
                       reason="delay h load until the hT stream has a head start")
        for b in range(NB):
            nc.scalar.activation(ET[:, ds(b * IB, IB)], sps[b][:], EXP, bias=uwu[:])

        # ---- ET re-transposes (one PSUM bank) + per-block max/zsum ----
        zcol = ps.tile([P, NT], f32, tag="acc", bufs=1)
        et = ps.tile([P, NT * P], bf16, tag="tp", bufs=1)
        for b in range(NB):
            for q in range(TPB):
                t = b * TPB + q
                nc.tensor.transpose(et[:, ds(t * P, P)], ET[:, ds(t * P, P)], ident[:])
            for q in range(TPB):
                t = b * TPB + q
                nc.tensor.matmul(
                    zcol[:, ds(t, 1)], ET[:, ds(t * P, P)], ones_col[:],
                    start=True, stop=True, skip_group_check=True,
                )
            sl = ds(b * TPB, TPB)
            nc.vector.reciprocal(z_rec[:, sl], zcol[:, sl])
            et3 = et[:].rearrange("p (t x) -> p t x", t=NT)
            nc.vector.reduce_max(m_exp[:, sl], et3[:, ds(b * TPB, TPB), :], axis=AX)
            nc.gpsimd.tensor_copy(m_bf[:, sl], m_exp[:, sl])

        # ---- passthrough (slab 0) per half as h lands ----
        for b in range(NB):
            nc.gpsimd.dma_start(
                out_d[:, ds(b * TPB * D, TPB * D)], h_all[:, ds(b * TPB * D, TPB * D)]
            )

        # ---- c2q u_a b0 (4-deep PSUM rotation); q2c hap/zq; u_a b1 ----
        up_tags = ["ua", "ua", "s0", "s0", "ua", "ua", "ux", "s0"]
        ups = []
        for t in range(TPB):
            up = ps.tile([P, D], f32, tag=up_tags[t], name=f"up{t}")
            nc.tensor.matmul(up[:], ET[:, ds(t * P, P)], u_sb, start=True, stop=True)
            ups.append(up)
        for t in range(NT):
            nc.tensor.matmul(
                hap[:], m_bf[:, ds(t, 1)], h_all[:, ds(t * D, D)],
                start=(t == 0), stop=(t == NT - 1), skip_group_check=True,
            )
        with tc.high_priority():
            mrow = consts.tile([P, 1], f32)
            nc.vector.reduce_sum(mrow[:], m_exp[:], axis=AX)
            mrow_bf = consts.tile([P, 1], bf16)
            nc.gpsimd.tensor_copy(mrow_bf[:], mrow[:])
            zqp = ps.tile([1, 1], f32, tag="acc", bufs=1)
            nc.tensor.matmul(zqp[:], mrow_bf[:], ones_col[:], start=True, stop=True)
        for t in range(TPB, NT):
            up = ps.tile([P, D], f32, tag=up_tags[t], name=f"up{t}", bufs=1 if t == 6 else 2)
            nc.tensor.matmul(up[:], ET[:, ds(t * P, P)], u_sb, start=True, stop=True)
            ups.append(up)

        # ---- evictions + q2c tail, FIFO-choreographed per engine ----
        def o2_of(t):
            b, q = divmod(t, TPB)
            return ua_blk[b][:, ds(q * D, D)]

        def o3_pair(pair):
            b, qp = divmod(pair, TPB // 2)
            nc.vector.tensor_mul(
                o3_blk[b][:, ds(qp * 2 * D, 2 * D)],
                ua_blk[b][:, ds(qp * 2 * D, 2 * D)],
                h_all[:, ds(pair * 2 * D, 2 * D)],
            )

        # scalar: o2 t0, t1 first; vector: mrow already queued
        nc.scalar.mul(o2_of(0), ups[0][:], z_rec[:, ds(0, 1)])
        nc.scalar.mul(o2_of(1), ups[1][:], z_rec[:, ds(1, 1)])
        o3_pair(0)                       # vector, fires when o2 t1 done
        with tc.high_priority():
            rzq = consts.tile([1, 1], f32)
            nc.vector.reciprocal(rzq[:], zqp[:])
            ha_row = consts.tile([1, D], bf16)
            nc.scalar.mul(ha_row[:], hap[:], rzq[:])
            haT = ps.tile([P, KC], f32, tag="acc", bufs=1)
            for k in range(KC):
                nc.tensor.matmul(
                    haT[:, ds(k, 1)], ha_row[:, ds(k * P, P)], one1[:],
                    start=True, stop=True, skip_group_check=True,
                )
        nc.scalar.mul(o2_of(2), ups[2][:], z_rec[:, ds(2, 1)])
        nc.scalar.mul(o2_of(3), ups[3][:], z_rec[:, ds(3, 1)])
        nc.vector.tensor_scalar_mul(o2_of(5), ups[5][:], z_rec[:, ds(5, 1)])
        nc.vector.tensor_scalar_mul(o2_of(7), ups[7][:], z_rec[:, ds(7, 1)])
        o3_pair(1)                       # vector
        nc.sync.dma_start(out_d[:, ds(NT * D, TPB * D)], ua_blk[0][:])
        nc.sync.dma_start(out_d[:, ds(2 * NT * D, TPB * D)], o3_blk[0][:])
        nc.scalar.mul(o2_of(4), ups[4][:], z_rec[:, ds(4, 1)])
        nc.scalar.mul(o2_of(6), ups[6][:], z_rec[:, ds(6, 1)])
        # vector: hacol then o4T (4x tensor_scalar, 3D APs) then o3 b1
        with tc.high_priority():
            hacol = consts.tile([P, KC], f32)
            nc.vector.tensor_copy(hacol[:], haT[:])
            hT4 = hT[:].rearrange("p (b k x) -> p b k x", b=NB, k=KC)
            o4T4 = o4T[:].rearrange("p (k b x) -> p k b x", k=KC, b=NB)
            for k in range(KC):
                nc.vector.tensor_scalar_mul(
                    o4T4[:, k], hT4[:, :, k, :], hacol[:, ds(k, 1)]
                )
            nc.gpsimd.dma_start(out_d[:, ds(3 * NT * D, 2 * JX)], o4T[:, ds(0, 2 * JX)])
            nc.gpsimd.dma_start(
                out_d[:, ds(3 * NT * D + 2 * JX, 2 * JX)], o4T[:, ds(2 * JX, 2 * JX)]
            )
        o3_pair(2)                       # vector
        nc.sync.dma_start(out_d[:, ds((NT + TPB) * D, TPB * D)], ua_blk[1][:])
        o3_pair(3)                       # vector
        nc.sync.dma_start(out_d[:, ds((2 * NT + TPB) * D, TPB * D)], o3_blk[1][:])

    nc.compile()
    return nc


def _get_nc():
    if "nc" not in _CACHE:
        _CACHE["nc"] = _build_program()
    return _CACHE["nc"]


def _ensure_axon_hooks_stub():
    import sys
    import types

    try:
        import antenv.axon_hooks  # noqa: F401
    except ImportError:
        mod = types.ModuleType("antenv.axon_hooks")
        _hook = [None]
        mod.set_axon_ntff_profile_hook = lambda hook: _hook.__setitem__(0, hook)
        mod.get_axon_ntff_profile_hook = lambda: _hook[0]
        sys.modules["antenv.axon_hooks"] = mod


def _prep_inputs(h, u, alpha_w):
    """Host-side layout/weight prep (data movement + O(JQ*D) weight folding)."""
    import ml_dtypes

    bf = ml_dtypes.bfloat16
    w_h, w_u, w_hu = alpha_w[:D], alpha_w[D:2 * D], alpha_w[2 * D:]
    in_maps = []
    for n in range(N_B):
        hn = h[n]                                   # [JX, D] f32
        un = u[n]                                   # [JQ, D] f32
        hrows = np.ascontiguousarray(
            hn.reshape(NT, P, D).transpose(1, 0, 2).reshape(P, NT * D)
        ).astype(bf)
        # block-major hT: [P, (b, k, i')]
        hT = np.ascontiguousarray(
            hn.T.reshape(KC, P, NB, IB).transpose(1, 2, 0, 3).reshape(P, NB * KC * IB)
        ).astype(bf)
        uwb = un * w_hu[None, :] + w_h[None, :]     # [JQ, D]
        uwbT = uwb.T.reshape(KC, P, JQ).transpose(1, 0, 2).reshape(P, KC * JQ)
        aux = np.concatenate([un, uwbT], axis=1).astype(bf)
        uwu = (un @ w_u).reshape(P, 1).astype(np.float32)
        in_maps.append({"hrows": hrows, "hT": hT, "aux": np.ascontiguousarray(aux),
                        "uwu": uwu})
    return in_maps


def _decode_out(res):
    outs = []
    for n in range(N_B):
        o = np.asarray(res.results[n]["out"]).astype(np.float32)
        slabs = o.reshape(P, 4, NT * D)
        rows = slabs[:, :3, :].reshape(P, 3, NT, D).transpose(2, 0, 1, 3)  # [NT,P,3,D]
        o4 = slabs[:, 3, :].reshape(P, KC, JX).transpose(2, 1, 0)          # [JX,KC,P]
        full = np.concatenate(
            [rows.reshape(JX, 3 * D), o4.reshape(JX, D)], axis=1
        )
        outs.append(full)
    return np.stack(outs, axis=0).reshape(N_B, M_B, JX, 4 * D)


def kernel(h, u, alpha_w, alpha_b=None, **_unused):
    _ensure_axon_hooks_stub()
    from concourse.bass_utils import run_bass_kernel_spmd

    h = np.ascontiguousarray(np.asarray(h, dtype=np.float32)).reshape(N_B, JX, D)
    u = np.ascontiguousarray(np.asarray(u, dtype=np.float32)).reshape(N_B, JQ, D)
    alpha_w = np.ascontiguousarray(np.asarray(alpha_w, dtype=np.float32)).reshape(3 * D)

    nc = _get_nc()
    in_maps = _prep_inputs(h, u, alpha_w)
    res = run_bass_kernel_spmd(nc, in_maps, core_ids=list(range(N_B)))
    return _decode_out(res)
